# revision 2
# baseline (speedup 1.0000x reference)
"""Trainium2 Bass kernel for a 4-step differentiable recurrent net forward pass.

Reference computation (B=8192, NI=512, NH=2048, NO=512, 4 steps):
    activs = 0; outputs = 0
    repeat 4x:  pre = hr * (x @ Wih.T + activs @ Whh.T + outputs @ Woh.T) + hb
                activs = per_neuron_act(pre)        # tanh/sigmoid/relu by i%3
    out = sigmoid(or * (x @ Wio.T + outputs @ Woo.T + activs @ Who.T) + ob)

`outputs` is never written inside the loop, so the Woh/Woo terms vanish and
the x-projection P = hr*(x@Wih.T)+hb is loop-invariant (computed once).

Strategy: data-parallel on batch across 8 cores (1024 rows each). On-core
everything is feature-major (features on SBUF partitions, batch on the free
axis), so each matmul is W_tile.T @ X^T with stationary weights. All matmuls
run in fp8 e4m3 with DoubleRow perf mode (two k-tiles per instruction, 2x
MAC throughput). Weights are scaled by S=256 host-side so their ~0.02-scale
values sit in e4m3's normal range; the 1/S is folded into the activation
instruction's input scale. Activations are quantized to e4m3 unscaled (they
are O(1)). PSUM accumulates in fp32 throughout, so only operand quantization
loses precision (~1.3e-2 rel err on the final sigmoid outputs).

The PE streams fp8-DR matmuls at ~216ns per 512-column instruction (1
col/cycle @2.4GHz), which puts the 920-matmul schedule at a ~198us floor;
everything else is about keeping the PE fed:
  - every weight matrix is host-packed into per-(m-block) pieces that are
    contiguous per partition on BOTH the DRAM and SBUF side, so DMA
    descriptors are 1-8KB and the descriptor feed never caps the queue
    (512B-strided pieces previously capped the sync queue at ~110GB/s,
    starving the first hh step for ~11us);
  - pieces are spread across three trigger paths (SP HWDGE queue for
    x+wih+who, gpsimd software DGE for the 4MB whh in four 1MB
    contiguous mb-pieces, ACT queue for the small operands) in exact
    consumption order;
  - the PE p-state/HAM ramp is absorbed by dummy warmup matmuls gated
    only on a DVE memset (the DVE is alive ~1us before gpsimd), so the
    array is at full clock roughly when the first real operands land.
Host-side prep: hidden neurons are permuted so the three activation groups
are contiguous, hr/or are folded into the weight matrices, and hb/ob are
applied as per-partition bias APs inside the ACT instructions.
"""

import os

import numpy as np
import ml_dtypes

import concourse.bass as bass
import concourse.tile as tile
from concourse import bacc, mybir
from concourse.bass_utils import run_bass_kernel_spmd

B, NI, NH, NO = 8192, 512, 2048, 512
N_STEPS = 4
N_CORES = 8
BL = B // N_CORES          # batch rows per core
CH = 512                   # batch chunk (one PSUM bank of fp32)
NCH = BL // CH             # 2 chunks per core
KI = NI // 128             # 4 k-tiles over inputs
KH = NH // 128             # 16 k/m-tiles over hidden
KO = NO // 128             # 4 m-tiles over outputs
NB = KH // 4               # 4 m-blocks of 4 m-tiles over hidden

FP8 = mybir.dt.float8e4
BF16 = mybir.dt.bfloat16
F32 = mybir.dt.float32
AF = mybir.ActivationFunctionType
DR = mybir.MatmulPerfMode.DoubleRow
E4 = ml_dtypes.float8_e4m3

WS = 256.0                 # weight scale into fp8 range
IWS = 1.0 / WS             # folded back out at activation time

# hidden neurons regrouped as [all tanh | all sigmoid | all relu]
_idx = np.arange(NH)
PERM = np.concatenate([_idx[_idx % 3 == 0], _idx[_idx % 3 == 1], _idx[_idx % 3 == 2]])
_B1 = int((_idx % 3 == 0).sum())           # 683
_B2 = _B1 + int((_idx % 3 == 1).sum())     # 1366

# per m-tile: the single activation function, or None for the two mixed tiles
_TILE_FUNC = []
for _m in range(KH):
    _lo, _hi = _m * 128, (_m + 1) * 128
    _fs = set()
    for _f, _a, _b in ((AF.Tanh, 0, _B1), (AF.Sigmoid, _B1, _B2), (AF.Relu, _B2, NH)):
        if max(_lo, _a) < min(_hi, _b):
            _fs.add(_f)
    _TILE_FUNC.append(_fs.pop() if len(_fs) == 1 else None)

# mixed tiles: (major_func applied everywhere, minor_func, mask column block)
# partition sub-ranges must be 32-aligned on TRN2, so the minority strip is
# fixed up with a full-tile ACT + copy_predicated against a {0,1} mask
_BOUNDARY = {
    _B1 // 128: (AF.Sigmoid, AF.Tanh, 0),    # tile 5: parts < 43 are tanh
    _B2 // 128: (AF.Sigmoid, AF.Relu, 1),    # tile 10: parts >= 86 are relu
}


def _emit_hidden_act(nc, ps, blk, a_new, tmp_pool, bmask_t, hbc_t):
    """Run a 4-m-tile block of WS-scaled pre-activations through the grouped
    activations into a_new, applying the raw hidden bias inside the ACT.

    ps:    AP (128, 4*CH) holding m-tiles blk*4..blk*4+3 side by side
    a_new: SBUF tile (128, KH, CH) fp8, m-tile m lives at [:, m, :]
    hbc_t: (128, KH) f32 per-partition raw biases, column m for m-tile m
    """
    for mloc in range(4):
        m = blk * 4 + mloc
        bias = hbc_t[:, m:m + 1]
        src = ps[:, mloc * CH:(mloc + 1) * CH]
        if m in _BOUNDARY:
            major, minor, mb = _BOUNDARY[m]
            nc.scalar.activation(a_new[:, m:m + 1, :], src, major,
                                 bias=bias, scale=IWS)
            t = tmp_pool.tile([128, CH], FP8, tag="btmp", bufs=2, name="btmp")
            nc.scalar.activation(t[:], src, minor, bias=bias, scale=IWS)
            nc.vector.copy_predicated(
                a_new[:, m:m + 1, :],
                bmask_t[:, mb * CH:(mb + 1) * CH], t[:])
        else:
            nc.scalar.activation(a_new[:, m:m + 1, :], src, _TILE_FUNC[m],
                                 bias=bias, scale=IWS)


def _build_nc():
    nc = bacc.Bacc("TRN2", target_bir_lowering=False, debug=False,
                   num_devices=N_CORES, dynamic_dma_scratch_size=2048)

    # All operands are host-packed into pieces that are contiguous per
    # partition on both the DRAM and SBUF side, so every DMA descriptor is
    # 1KB+ (the HWDGE descriptor feed caps a queue at ~110GB/s with 512B
    # descriptors, but the 16 shared DMA engines sustain ~350GB/s with
    # 2-8KB ones).
    #   xT:   [128, c*KI+kt, ch]      piece (c,kp) = 1KB/partition
    #   wihp: [mb*128+p, kt, col]     piece (mb)   = 2KB/partition
    #   whhp: [mb*128+p, kt, col]     piece (mb)   = 8KB/partition
    #   whop: [mo*128+p, kt, col]     piece (mo)   = 2KB/partition
    xT = nc.dram_tensor("xT", [128, NCH * KI, CH], FP8,
                        kind="ExternalInput").ap()
    wihp = nc.dram_tensor("wihp", [NB * 128, KI, 512], FP8,
                          kind="ExternalInput").ap()
    whhp = nc.dram_tensor("whhp", [NB * 128, KH, 512], FP8,
                          kind="ExternalInput").ap()
    whop = nc.dram_tensor("whop", [KO * 128, KH, 128], FP8,
                          kind="ExternalInput").ap()
    wio = nc.dram_tensor("wio", [128, KI, NO], FP8, kind="ExternalInput").ap()
    hbc = nc.dram_tensor("hbc", [128, KH], F32, kind="ExternalInput").ap()
    obc = nc.dram_tensor("obc", [128, KO], F32, kind="ExternalInput").ap()
    bmask = nc.dram_tensor("bmask", [128, 2 * CH], mybir.dt.uint8,
                           kind="ExternalInput").ap()
    outT = nc.dram_tensor("outT", [NO, BL], BF16, kind="ExternalOutput").ap()

    with tile.TileContext(nc) as tc:
        with tc.tile_pool(name="w", bufs=1) as wpool, \
             tc.tile_pool(name="act", bufs=1) as apool, \
             tc.tile_pool(name="ps", bufs=2, space="PSUM") as pspool, \
             tc.tile_pool(name="out", bufs=4) as opool:

            wih_t = [wpool.tile([128, KI, 512], FP8, tag=f"wih{mb}",
                                name=f"wih{mb}") for mb in range(NB)]
            whh_t = [wpool.tile([128, KH, 512], FP8, tag=f"whh{mb}",
                                name=f"whh{mb}") for mb in range(NB)]
            who_t = [wpool.tile([128, KH, 128], FP8, tag=f"who{mo}",
                                name=f"who{mo}") for mo in range(KO)]
            x_m = wpool.tile([128, NCH * KI, CH], FP8, tag="x", name="xm")
            wio_m = wpool.tile([128, KI, NO], FP8, tag="wio", name="wiom")
            hbc_t = wpool.tile([128, KH], F32, tag="hbc")
            obc_t = wpool.tile([128, KO], F32, tag="obc")
            bmask_t = wpool.tile([128, 2 * CH], mybir.dt.uint8, tag="bmask")

            # ---- PE warmup: dummy matmuls gated only on a DVE memset (the
            # DVE sequencer comes alive ~1us before gpsimd), so the p-state
            # ramp and HAM clock-gate run against garbage work while the
            # first real operands are still in flight (~10.3us). 12 x 256
            # columns spans ~3us of PE time from a ~7.4us start. ----
            warm_t = wpool.tile([128, 2, 256], FP8, tag="warm", name="warm")
            nc.vector.memset(warm_t[:], 0.0)
            ps_w = pspool.tile([128, 4 * CH], F32, tag="ps", name="psw")
            for _w in range(12):
                nc.tensor.matmul(
                    ps_w[:, (_w % 4) * CH:(_w % 4) * CH + 256],
                    warm_t[:, :, 0:128], warm_t[:],
                    start=True, stop=True, perf_mode=DR,
                    skip_group_check=True)

            # ---- stage all inputs in exact consumption order ----
            # SP HWDGE queue: the x-projection stream, then who (needed only
            # at ~190us but the queue is otherwise idle).
            nc.sync.dma_start(x_m[:, 0:2, :], xT[:, 0:2, :])
            nc.sync.dma_start(wih_t[0][:], wihp[0:128])
            nc.sync.dma_start(x_m[:, 2:4, :], xT[:, 2:4, :])
            for mb in range(1, NB):
                nc.sync.dma_start(wih_t[mb][:], wihp[mb * 128:(mb + 1) * 128])
            nc.sync.dma_start(x_m[:, KI:KI + 2, :], xT[:, KI:KI + 2, :])
            nc.sync.dma_start(x_m[:, KI + 2:KI + 4, :],
                              xT[:, KI + 2:KI + 4, :])
            for mo in range(KO):
                nc.sync.dma_start(who_t[mo][:], whop[mo * 128:(mo + 1) * 128])
            # gpsimd software DGE: the 4MB whh as four contiguous 1MB
            # pieces, mb-major to match the hh-step block order (the
            # software DGE is only reliable for contiguous-per-partition
            # transfers, which these are).
            for mb in range(NB):
                nc.gpsimd.dma_start(whh_t[mb][:],
                                    whhp[mb * 128:(mb + 1) * 128])
            # ACT HWDGE queue: small early operands.
            nc.scalar.dma_start(hbc_t[:], hbc[:])
            nc.scalar.dma_start(bmask_t[:], bmask[:])
            nc.scalar.dma_start(wio_m[:], wio[:])
            nc.scalar.dma_start(obc_t[:], obc[:])

            # ---- per-chunk x-projection P and first-step activations ----
            P = {}
            A = {}
            for c in range(NCH):
                P[c] = apool.tile([128, KH * CH], BF16, tag=f"P{c}",
                                  name=f"P{c}")
                a1 = apool.tile([128, KH, CH], FP8, tag="A", bufs=3,
                                name=f"A1c{c}")
                for blk in range(4):
                    ps = pspool.tile([128, 4 * CH], F32, tag="ps", name="psb")
                    for kp in range(KI // 2):
                        for mloc in range(4):
                            nc.tensor.matmul(
                                ps[:, mloc * CH:(mloc + 1) * CH],
                                wih_t[blk][:, 2 * kp:2 * kp + 2,
                                           mloc * 128:(mloc + 1) * 128],
                                x_m[:, c * KI + 2 * kp:c * KI + 2 * kp + 2, :],
                                start=(kp == 0), stop=(kp == KI // 2 - 1),
                                perf_mode=DR)
                    # P holds the raw WS-scaled x-projection (bias is applied
                    # inside the ACTs); a single copy frees the PSUM slot
                    nc.vector.tensor_copy(
                        P[c][:, blk * 4 * CH:(blk + 1) * 4 * CH], ps[:])
                    _emit_hidden_act(nc, P[c][:, blk * 4 * CH:(blk + 1) * 4 * CH],
                                     blk, a1, opool, bmask_t, hbc_t)
                A[c] = a1

            # ---- whh-independent output x-projection (fills the window
            # while the whh load is still in flight) ----
            outx = {}
            for c in range(NCH):
                outx[c] = apool.tile([128, KO * CH], BF16, tag=f"outx{c}",
                                     name=f"outx{c}")
                ps = pspool.tile([128, 4 * CH], F32, tag="ps", name="psb")
                for kp in range(KI // 2):
                    for mo in range(KO):
                        nc.tensor.matmul(
                            ps[:, mo * CH:(mo + 1) * CH],
                            wio_m[:, 2 * kp:2 * kp + 2,
                                  mo * 128:(mo + 1) * 128],
                            x_m[:, c * KI + 2 * kp:c * KI + 2 * kp + 2, :],
                            start=(kp == 0), stop=(kp == KI // 2 - 1),
                            perf_mode=DR)
                nc.vector.tensor_copy(outx[c][:], ps[:])

            # ---- recurrent steps 2..4 ----
            def hh_step(c, s):
                a_new = apool.tile([128, KH, CH], FP8, tag="A", bufs=3,
                                   name=f"A{s + 2}c{c}")
                for blk in range(4):
                    ps = pspool.tile([128, 4 * CH], F32, tag="ps", name="psb")
                    for kp in range(KH // 2):
                        for mloc in range(4):
                            nc.tensor.matmul(
                                ps[:, mloc * CH:(mloc + 1) * CH],
                                whh_t[blk][:, 2 * kp:2 * kp + 2,
                                           mloc * 128:(mloc + 1) * 128],
                                A[c][:, 2 * kp:2 * kp + 2, :],
                                start=(kp == 0), stop=(kp == KH // 2 - 1),
                                perf_mode=DR)
                    # pre = psum + P into an SBUF temp: a single PSUM read
                    # frees the bank; ACT then runs off SBUF
                    tmp = opool.tile([128, 4 * CH], F32, tag="pre", bufs=2,
                                     name="pre")
                    nc.vector.tensor_add(
                        tmp[:], ps[:], P[c][:, blk * 4 * CH:(blk + 1) * 4 * CH])
                    _emit_hidden_act(nc, tmp, blk, a_new, opool, bmask_t,
                                     hbc_t)
                A[c] = a_new

            for s in range(N_STEPS - 2):
                for c in range(NCH):
                    hh_step(c, s)
            hh_step(0, N_STEPS - 2)  # chunk 1's final step emitted after

            # ---- output layer; chunk 0's output post-chain overlaps chunk
            # 1's final hh step and output matmuls ----
            def out_chunk(c):
                last = (c == NCH - 1)
                for mo in range(KO):
                    pso = pspool.tile([128, CH], F32, tag="ps", name="pso")
                    oap = pso[:]
                    for kp in range(KH // 2):
                        nc.tensor.matmul(
                            oap,
                            who_t[mo][:, 2 * kp:2 * kp + 2, :],
                            A[c][:, 2 * kp:2 * kp + 2, :],
                            start=(kp == 0), stop=(kp == KH // 2 - 1),
                            perf_mode=DR)
                    to = opool.tile([128, CH], F32, tag="preo", bufs=2,
                                    name="preo")
                    nc.vector.tensor_add(
                        to[:], oap, outx[c][:, mo * CH:(mo + 1) * CH])
                    o = opool.tile([128, CH], BF16, tag="o", bufs=2, name="o")
                    if last and mo == KO - 1:
                        # split the very last tile so its ACT and store
                        # pipeline instead of trailing the final matmul
                        for h, eng in ((0, nc.sync), (1, nc.scalar)):
                            nc.scalar.activation(
                                o[:, h * 256:(h + 1) * 256],
                                to[:, h * 256:(h + 1) * 256], AF.Sigmoid,
                                bias=obc_t[:, mo:mo + 1], scale=IWS)
                            eng.dma_start(
                                outT[mo * 128:(mo + 1) * 128,
                                     c * CH + h * 256:c * CH + (h + 1) * 256],
                                o[:, h * 256:(h + 1) * 256])
                    else:
                        nc.scalar.activation(o[:], to[:], AF.Sigmoid,
                                             bias=obc_t[:, mo:mo + 1],
                                             scale=IWS)
                        eng = nc.sync if mo % 2 == 0 else nc.scalar
                        eng.dma_start(
                            outT[mo * 128:(mo + 1) * 128,
                                 c * CH:(c + 1) * CH],
                            o[:])

            hh_step(1, N_STEPS - 2)
            out_chunk(0)
            out_chunk(1)

    nc.compile()
    return nc


_NC_CACHE = None


def _get_nc():
    global _NC_CACHE
    if _NC_CACHE is None:
        _NC_CACHE = _build_nc()
    return _NC_CACHE


def _make_bmask():
    m = np.zeros((128, 2 * CH), np.uint8)
    m[:_B1 - (_B1 // 128) * 128, 0:CH] = 1          # tile 5: parts < 43 tanh
    m[_B2 - (_B2 // 128) * 128:, CH:2 * CH] = 1     # tile 10: parts >= 86 relu
    return m


def _q8(a):
    return np.clip(a, -240.0, 240.0).astype(E4)


def _prep_in_maps(inputs):
    x = np.asarray(inputs["inputs"], np.float32)
    hr = np.asarray(inputs["hidden_responses"], np.float32)[PERM]
    hb = np.asarray(inputs["hidden_biases"], np.float32)[PERM]
    orr = np.asarray(inputs["output_responses"], np.float32)
    ob = np.asarray(inputs["output_biases"], np.float32)

    wih_s = WS * (hr[:, None] * np.asarray(inputs["input_to_hidden"], np.float32)[PERM]).T
    whh_s = WS * (hr[:, None] *
                  np.asarray(inputs["hidden_to_hidden"], np.float32)[PERM][:, PERM]).T
    who_s = WS * (orr[:, None] *
                  np.asarray(inputs["hidden_to_output"], np.float32)[:, PERM]).T
    wio_s = WS * (orr[:, None] * np.asarray(inputs["input_to_output"], np.float32)).T

    def pack(w, ktiles):     # (ktiles*128, C) -> (128, ktiles, C)
        c = w.shape[1]
        return np.ascontiguousarray(
            w.reshape(ktiles, 128, c).transpose(1, 0, 2))

    def pack_mb(w, ktiles, mw):
        # (ktiles*128, C) -> (C//mw blocks)[128, ktiles, mw] stacked on dim0:
        # piece mb holds columns mb*mw..(mb+1)*mw, contiguous per partition
        p = pack(w, ktiles)                       # (128, ktiles, C)
        nmb = p.shape[2] // mw
        return np.ascontiguousarray(
            p.reshape(128, ktiles, nmb, mw).transpose(2, 0, 1, 3)
            .reshape(nmb * 128, ktiles, mw))

    shared = {
        "wihp": _q8(pack_mb(wih_s, KI, 512)),
        "whhp": _q8(pack_mb(whh_s, KH, 512)),
        "whop": _q8(pack_mb(who_s, KH, 128)),
        "wio": _q8(pack(wio_s, KI)),
        "hbc": np.ascontiguousarray(hb.reshape(KH, 128).T),
        "obc": np.ascontiguousarray(ob.reshape(KO, 128).T),
        "bmask": _make_bmask(),
    }
    in_maps = []
    for c in range(N_CORES):
        m = dict(shared)
        xp = pack(np.ascontiguousarray(x[c * BL:(c + 1) * BL].T), KI)
        m["xT"] = _q8(np.ascontiguousarray(
            xp.reshape(128, KI, NCH, CH).transpose(0, 2, 1, 3)
            .reshape(128, NCH * KI, CH)))
        in_maps.append(m)
    return in_maps


def _run(inputs, trace=False, tmpdir=None):
    nc = _get_nc()
    in_maps = _prep_in_maps(inputs)
    res = run_bass_kernel_spmd(nc, in_maps, core_ids=list(range(N_CORES)),
                               trace=trace, tmpdir=tmpdir)
    out = np.empty((B, NO), np.float32)
    for c in range(N_CORES):
        out[c * BL:(c + 1) * BL] = res.results[c]["outT"].T.astype(np.float32)
    return out, res


def kernel(**inputs) -> np.ndarray:
    out, _ = _run(inputs, trace=False)
    return out


if __name__ == "__main__":
    rng = np.random.default_rng(0)
    ins = {
        "inputs": rng.standard_normal((B, NI), dtype=np.float32),
        "input_to_hidden": rng.standard_normal((NH, NI), dtype=np.float32) * 0.02,
        "hidden_to_hidden": rng.standard_normal((NH, NH), dtype=np.float32) * 0.02,
        "output_to_hidden": rng.standard_normal((NH, NO), dtype=np.float32) * 0.02,
        "input_to_output": rng.standard_normal((NO, NI), dtype=np.float32) * 0.02,
        "hidden_to_output": rng.standard_normal((NO, NH), dtype=np.float32) * 0.02,
        "output_to_output": rng.standard_normal((NO, NO), dtype=np.float32) * 0.02,
        "hidden_responses": rng.standard_normal(NH, dtype=np.float32) * 0.1 + 1.0,
        "hidden_biases": rng.standard_normal(NH, dtype=np.float32) * 0.1,
        "output_responses": rng.standard_normal(NO, dtype=np.float32) * 0.1 + 1.0,
        "output_biases": rng.standard_normal(NO, dtype=np.float32) * 0.1,
    }
    out = kernel(**ins)
    print("kernel output", out.shape, out.dtype, out[:2, :4])


# revision 3
# speedup vs baseline: 1.0516x; 1.0516x over previous
"""Trainium2 Bass kernel for a 4-step differentiable recurrent net forward pass.

Reference computation (B=8192, NI=512, NH=2048, NO=512, 4 steps):
    activs = 0; outputs = 0
    repeat 4x:  pre = hr * (x @ Wih.T + activs @ Whh.T + outputs @ Woh.T) + hb
                activs = per_neuron_act(pre)        # tanh/sigmoid/relu by i%3
    out = sigmoid(or * (x @ Wio.T + outputs @ Woo.T + activs @ Who.T) + ob)

`outputs` is never written inside the loop, so the Woh/Woo terms vanish and
the x-projection P = hr*(x@Wih.T)+hb is loop-invariant (computed once).

Strategy: data-parallel on batch across 8 cores (1024 rows each). On-core
everything is feature-major (features on SBUF partitions, batch on the free
axis), so each matmul is W_tile.T @ X^T with stationary weights. All matmuls
run in fp8 e4m3 with DoubleRow perf mode (two k-tiles per instruction, 2x
MAC throughput). Weights are scaled by S=256 host-side so their ~0.02-scale
values sit in e4m3's normal range; the 1/S is folded into the activation
instruction's input scale. Activations are quantized to e4m3 unscaled (they
are O(1)). PSUM accumulates in fp32 throughout, so only operand quantization
loses precision (~1.3e-2 rel err on the final sigmoid outputs).

The PE streams fp8-DR matmuls at ~216ns per 512-column instruction (1
col/cycle @2.4GHz), which puts the 920-matmul schedule at a ~198us floor;
everything else is about keeping the PE fed:
  - every weight matrix is host-packed into per-(m-block) pieces that are
    contiguous per partition on BOTH the DRAM and SBUF side, so DMA
    descriptors are 1-8KB and the descriptor feed never caps the queue
    (512B-strided pieces previously capped the sync queue at ~110GB/s,
    starving the first hh step for ~11us);
  - pieces are spread across three trigger paths (SP HWDGE queue for
    x+wih+who, gpsimd software DGE for the 4MB whh in four 1MB
    contiguous mb-pieces, ACT queue for the small operands) in exact
    consumption order;
  - the PE p-state/HAM ramp is absorbed by dummy warmup matmuls gated
    only on a DVE memset (the DVE is alive ~1us before gpsimd), so the
    array is at full clock roughly when the first real operands land.
Host-side prep: hidden neurons are permuted so the three activation groups
are contiguous, hr/or are folded into the weight matrices, and hb/ob are
applied as per-partition bias APs inside the ACT instructions.
"""

import os

import numpy as np
import ml_dtypes

import concourse.bass as bass
import concourse.tile as tile
from concourse import bacc, mybir
from concourse.bass_utils import run_bass_kernel_spmd

B, NI, NH, NO = 8192, 512, 2048, 512
N_STEPS = 4
N_CORES = 8
BL = B // N_CORES          # batch rows per core
CH = 512                   # batch chunk (one PSUM bank of fp32)
NCH = BL // CH             # 2 chunks per core
KI = NI // 128             # 4 k-tiles over inputs
KH = NH // 128             # 16 k/m-tiles over hidden
KO = NO // 128             # 4 m-tiles over outputs
NB = KH // 4               # 4 m-blocks of 4 m-tiles over hidden

FP8 = mybir.dt.float8e4
BF16 = mybir.dt.bfloat16
F32 = mybir.dt.float32
AF = mybir.ActivationFunctionType
DR = mybir.MatmulPerfMode.DoubleRow
E4 = ml_dtypes.float8_e4m3

WS = 256.0                 # weight scale into fp8 range
IWS = 1.0 / WS             # folded back out at activation time

# hidden neurons regrouped as [all tanh | all sigmoid | all relu]
_idx = np.arange(NH)
PERM = np.concatenate([_idx[_idx % 3 == 0], _idx[_idx % 3 == 1], _idx[_idx % 3 == 2]])
_B1 = int((_idx % 3 == 0).sum())           # 683
_B2 = _B1 + int((_idx % 3 == 1).sum())     # 1366

# per m-tile: the single activation function, or None for the two mixed tiles
_TILE_FUNC = []
for _m in range(KH):
    _lo, _hi = _m * 128, (_m + 1) * 128
    _fs = set()
    for _f, _a, _b in ((AF.Tanh, 0, _B1), (AF.Sigmoid, _B1, _B2), (AF.Relu, _B2, NH)):
        if max(_lo, _a) < min(_hi, _b):
            _fs.add(_f)
    _TILE_FUNC.append(_fs.pop() if len(_fs) == 1 else None)

# mixed tiles: (major_func applied everywhere, minor_func, mask column block)
# partition sub-ranges must be 32-aligned on TRN2, so the minority strip is
# fixed up with a full-tile ACT + copy_predicated against a {0,1} mask
_BOUNDARY = {
    _B1 // 128: (AF.Sigmoid, AF.Tanh, 0),    # tile 5: parts < 43 are tanh
    _B2 // 128: (AF.Sigmoid, AF.Relu, 1),    # tile 10: parts >= 86 are relu
}


def _emit_hidden_act(nc, ps, blk, a_new, tmp_pool, bmask_t, hbc_t):
    """Run a 4-m-tile block of WS-scaled pre-activations through the grouped
    activations into a_new, applying the raw hidden bias inside the ACT.

    ps:    AP (128, 4*CH) holding m-tiles blk*4..blk*4+3 side by side
    a_new: SBUF tile (128, KH, CH) fp8, m-tile m lives at [:, m, :]
    hbc_t: (128, KH) f32 per-partition raw biases, column m for m-tile m
    """
    for mloc in range(4):
        m = blk * 4 + mloc
        bias = hbc_t[:, m:m + 1]
        src = ps[:, mloc * CH:(mloc + 1) * CH]
        if m in _BOUNDARY:
            major, minor, mb = _BOUNDARY[m]
            nc.scalar.activation(a_new[:, m:m + 1, :], src, major,
                                 bias=bias, scale=IWS)
            t = tmp_pool.tile([128, CH], FP8, tag="btmp", bufs=2, name="btmp")
            nc.scalar.activation(t[:], src, minor, bias=bias, scale=IWS)
            nc.vector.copy_predicated(
                a_new[:, m:m + 1, :],
                bmask_t[:, mb * CH:(mb + 1) * CH], t[:])
        else:
            nc.scalar.activation(a_new[:, m:m + 1, :], src, _TILE_FUNC[m],
                                 bias=bias, scale=IWS)


def _build_nc():
    nc = bacc.Bacc("TRN2", target_bir_lowering=False, debug=False,
                   num_devices=N_CORES, dynamic_dma_scratch_size=2048)

    # All operands are host-packed into pieces that are contiguous per
    # partition on both the DRAM and SBUF side, so every DMA descriptor is
    # 1KB+ (the HWDGE descriptor feed caps a queue at ~110GB/s with 512B
    # descriptors, but the 16 shared DMA engines sustain ~350GB/s with
    # 2-8KB ones).
    #   xT:   [128, c*KI+kt, ch]      piece (c,kp) = 1KB/partition
    #   wihp: [mb*128+p, kt, col]     piece (mb)   = 2KB/partition
    #   whhp: [mb*128+p, kt, col]     piece (mb)   = 8KB/partition
    #   whop: [mo*128+p, kt, col]     piece (mo)   = 2KB/partition
    xT = nc.dram_tensor("xT", [128, NCH * KI, CH], FP8,
                        kind="ExternalInput").ap()
    wihp = nc.dram_tensor("wihp", [NB * 128, KI, 512], FP8,
                          kind="ExternalInput").ap()
    whhp = nc.dram_tensor("whhp", [NB * 128, KH, 512], FP8,
                          kind="ExternalInput").ap()
    whop = nc.dram_tensor("whop", [KO * 128, KH, 128], FP8,
                          kind="ExternalInput").ap()
    wio = nc.dram_tensor("wio", [128, KI, NO], FP8, kind="ExternalInput").ap()
    hbc = nc.dram_tensor("hbc", [128, KH], F32, kind="ExternalInput").ap()
    obc = nc.dram_tensor("obc", [128, KO], F32, kind="ExternalInput").ap()
    bmask = nc.dram_tensor("bmask", [128, 2 * CH], mybir.dt.uint8,
                           kind="ExternalInput").ap()
    outT = nc.dram_tensor("outT", [NO, BL], BF16, kind="ExternalOutput").ap()

    with tile.TileContext(nc) as tc:
        with tc.tile_pool(name="w", bufs=1) as wpool, \
             tc.tile_pool(name="act", bufs=1) as apool, \
             tc.tile_pool(name="ps", bufs=2, space="PSUM") as pspool, \
             tc.tile_pool(name="out", bufs=4) as opool:

            wih_t = [wpool.tile([128, KI, 512], FP8, tag=f"wih{mb}",
                                name=f"wih{mb}") for mb in range(NB)]
            whh_t = [wpool.tile([128, KH, 512], FP8, tag=f"whh{mb}",
                                name=f"whh{mb}") for mb in range(NB)]
            who_t = [wpool.tile([128, KH, 128], FP8, tag=f"who{mo}",
                                name=f"who{mo}") for mo in range(KO)]
            x_m = wpool.tile([128, NCH * KI, CH], FP8, tag="x", name="xm")
            wio_m = wpool.tile([128, KI, NO], FP8, tag="wio", name="wiom")
            hbc_t = wpool.tile([128, KH], F32, tag="hbc")
            obc_t = wpool.tile([128, KO], F32, tag="obc")
            bmask_t = wpool.tile([128, 2 * CH], mybir.dt.uint8, tag="bmask")

            # ---- PE warmup: dummy matmuls gated only on a DVE memset (the
            # DVE sequencer comes alive ~1us before gpsimd), so the p-state
            # ramp and HAM clock-gate run against garbage work while the
            # first real operands are still in flight (~10.3us). 12 x 256
            # columns spans ~3us of PE time from a ~7.4us start. ----
            warm_t = wpool.tile([128, 2, 256], FP8, tag="warm", name="warm")
            nc.vector.memset(warm_t[:], 0.0)
            ps_w = pspool.tile([128, 4 * CH], F32, tag="ps", name="psw")
            for _w in range(12):
                nc.tensor.matmul(
                    ps_w[:, (_w % 4) * CH:(_w % 4) * CH + 256],
                    warm_t[:, :, 0:128], warm_t[:],
                    start=True, stop=True, perf_mode=DR,
                    skip_group_check=True)

            # ---- stage all inputs in exact consumption order ----
            # ALL large operands go on the single SP HWDGE queue: with 1-8KB
            # descriptors one queue saturates the 16 shared DMA engines
            # (~360GB/s measured), and strict FIFO makes arrival order equal
            # consumption order. (A parallel gpsimd software-DGE stream for
            # whh was tried and starved the x/wih stream it was supposed to
            # overlap with — the engines favor the software queue.)
            nc.sync.dma_start(x_m[:, 0:2, :], xT[:, 0:2, :])
            nc.sync.dma_start(wih_t[0][:], wihp[0:128])
            nc.sync.dma_start(x_m[:, 2:4, :], xT[:, 2:4, :])
            for mb in range(1, NB):
                nc.sync.dma_start(wih_t[mb][:], wihp[mb * 128:(mb + 1) * 128])
            nc.sync.dma_start(x_m[:, KI:KI + 2, :], xT[:, KI:KI + 2, :])
            nc.sync.dma_start(x_m[:, KI + 2:KI + 4, :],
                              xT[:, KI + 2:KI + 4, :])
            for mb in range(NB):
                nc.sync.dma_start(whh_t[mb][:],
                                  whhp[mb * 128:(mb + 1) * 128])
            for mo in range(KO):
                nc.sync.dma_start(who_t[mo][:], whop[mo * 128:(mo + 1) * 128])
            # ACT HWDGE queue: small early operands.
            nc.scalar.dma_start(hbc_t[:], hbc[:])
            nc.scalar.dma_start(bmask_t[:], bmask[:])
            nc.scalar.dma_start(wio_m[:], wio[:])
            nc.scalar.dma_start(obc_t[:], obc[:])

            # ---- per-chunk x-projection P and first-step activations ----
            P = {}
            A = {}
            for c in range(NCH):
                P[c] = apool.tile([128, KH * CH], BF16, tag=f"P{c}",
                                  name=f"P{c}")
                a1 = apool.tile([128, KH, CH], FP8, tag="A", bufs=3,
                                name=f"A1c{c}")
                for blk in range(4):
                    ps = pspool.tile([128, 4 * CH], F32, tag="ps", name="psb")
                    for kp in range(KI // 2):
                        for mloc in range(4):
                            nc.tensor.matmul(
                                ps[:, mloc * CH:(mloc + 1) * CH],
                                wih_t[blk][:, 2 * kp:2 * kp + 2,
                                           mloc * 128:(mloc + 1) * 128],
                                x_m[:, c * KI + 2 * kp:c * KI + 2 * kp + 2, :],
                                start=(kp == 0), stop=(kp == KI // 2 - 1),
                                perf_mode=DR)
                    # P holds the raw WS-scaled x-projection (bias is applied
                    # inside the ACTs); a single copy frees the PSUM slot
                    nc.vector.tensor_copy(
                        P[c][:, blk * 4 * CH:(blk + 1) * 4 * CH], ps[:])
                    _emit_hidden_act(nc, P[c][:, blk * 4 * CH:(blk + 1) * 4 * CH],
                                     blk, a1, opool, bmask_t, hbc_t)
                A[c] = a1

            # ---- whh-independent output x-projection (fills the window
            # while the whh load is still in flight) ----
            outx = {}
            for c in range(NCH):
                outx[c] = apool.tile([128, KO * CH], BF16, tag=f"outx{c}",
                                     name=f"outx{c}")
                ps = pspool.tile([128, 4 * CH], F32, tag="ps", name="psb")
                for kp in range(KI // 2):
                    for mo in range(KO):
                        nc.tensor.matmul(
                            ps[:, mo * CH:(mo + 1) * CH],
                            wio_m[:, 2 * kp:2 * kp + 2,
                                  mo * 128:(mo + 1) * 128],
                            x_m[:, c * KI + 2 * kp:c * KI + 2 * kp + 2, :],
                            start=(kp == 0), stop=(kp == KI // 2 - 1),
                            perf_mode=DR)
                nc.vector.tensor_copy(outx[c][:], ps[:])

            # ---- recurrent steps 2..4 ----
            def hh_step(c, s):
                a_new = apool.tile([128, KH, CH], FP8, tag="A", bufs=3,
                                   name=f"A{s + 2}c{c}")
                for blk in range(4):
                    ps = pspool.tile([128, 4 * CH], F32, tag="ps", name="psb")
                    for kp in range(KH // 2):
                        for mloc in range(4):
                            nc.tensor.matmul(
                                ps[:, mloc * CH:(mloc + 1) * CH],
                                whh_t[blk][:, 2 * kp:2 * kp + 2,
                                           mloc * 128:(mloc + 1) * 128],
                                A[c][:, 2 * kp:2 * kp + 2, :],
                                start=(kp == 0), stop=(kp == KH // 2 - 1),
                                perf_mode=DR)
                    # pre = psum + P into an SBUF temp: a single PSUM read
                    # frees the bank; ACT then runs off SBUF
                    tmp = opool.tile([128, 4 * CH], F32, tag="pre", bufs=2,
                                     name="pre")
                    nc.vector.tensor_add(
                        tmp[:], ps[:], P[c][:, blk * 4 * CH:(blk + 1) * 4 * CH])
                    _emit_hidden_act(nc, tmp, blk, a_new, opool, bmask_t,
                                     hbc_t)
                A[c] = a_new

            for s in range(N_STEPS - 2):
                for c in range(NCH):
                    hh_step(c, s)
            hh_step(0, N_STEPS - 2)  # chunk 1's final step emitted after

            # ---- output layer; chunk 0's output post-chain overlaps chunk
            # 1's final hh step and output matmuls ----
            def out_chunk(c):
                last = (c == NCH - 1)
                for mo in range(KO):
                    pso = pspool.tile([128, CH], F32, tag="ps", name="pso")
                    oap = pso[:]
                    for kp in range(KH // 2):
                        nc.tensor.matmul(
                            oap,
                            who_t[mo][:, 2 * kp:2 * kp + 2, :],
                            A[c][:, 2 * kp:2 * kp + 2, :],
                            start=(kp == 0), stop=(kp == KH // 2 - 1),
                            perf_mode=DR)
                    to = opool.tile([128, CH], F32, tag="preo", bufs=2,
                                    name="preo")
                    nc.vector.tensor_add(
                        to[:], oap, outx[c][:, mo * CH:(mo + 1) * CH])
                    o = opool.tile([128, CH], BF16, tag="o", bufs=2, name="o")
                    if last and mo == KO - 1:
                        # split the very last tile so its ACT and store
                        # pipeline instead of trailing the final matmul
                        for h, eng in ((0, nc.sync), (1, nc.scalar)):
                            nc.scalar.activation(
                                o[:, h * 256:(h + 1) * 256],
                                to[:, h * 256:(h + 1) * 256], AF.Sigmoid,
                                bias=obc_t[:, mo:mo + 1], scale=IWS)
                            eng.dma_start(
                                outT[mo * 128:(mo + 1) * 128,
                                     c * CH + h * 256:c * CH + (h + 1) * 256],
                                o[:, h * 256:(h + 1) * 256])
                    else:
                        nc.scalar.activation(o[:], to[:], AF.Sigmoid,
                                             bias=obc_t[:, mo:mo + 1],
                                             scale=IWS)
                        eng = nc.sync if mo % 2 == 0 else nc.scalar
                        eng.dma_start(
                            outT[mo * 128:(mo + 1) * 128,
                                 c * CH:(c + 1) * CH],
                            o[:])

            hh_step(1, N_STEPS - 2)
            out_chunk(0)
            out_chunk(1)

    nc.compile()
    return nc


_NC_CACHE = None


def _get_nc():
    global _NC_CACHE
    if _NC_CACHE is None:
        _NC_CACHE = _build_nc()
    return _NC_CACHE


def _make_bmask():
    m = np.zeros((128, 2 * CH), np.uint8)
    m[:_B1 - (_B1 // 128) * 128, 0:CH] = 1          # tile 5: parts < 43 tanh
    m[_B2 - (_B2 // 128) * 128:, CH:2 * CH] = 1     # tile 10: parts >= 86 relu
    return m


def _q8(a):
    return np.clip(a, -240.0, 240.0).astype(E4)


def _prep_in_maps(inputs):
    x = np.asarray(inputs["inputs"], np.float32)
    hr = np.asarray(inputs["hidden_responses"], np.float32)[PERM]
    hb = np.asarray(inputs["hidden_biases"], np.float32)[PERM]
    orr = np.asarray(inputs["output_responses"], np.float32)
    ob = np.asarray(inputs["output_biases"], np.float32)

    wih_s = WS * (hr[:, None] * np.asarray(inputs["input_to_hidden"], np.float32)[PERM]).T
    whh_s = WS * (hr[:, None] *
                  np.asarray(inputs["hidden_to_hidden"], np.float32)[PERM][:, PERM]).T
    who_s = WS * (orr[:, None] *
                  np.asarray(inputs["hidden_to_output"], np.float32)[:, PERM]).T
    wio_s = WS * (orr[:, None] * np.asarray(inputs["input_to_output"], np.float32)).T

    def pack(w, ktiles):     # (ktiles*128, C) -> (128, ktiles, C)
        c = w.shape[1]
        return np.ascontiguousarray(
            w.reshape(ktiles, 128, c).transpose(1, 0, 2))

    def pack_mb(w, ktiles, mw):
        # (ktiles*128, C) -> (C//mw blocks)[128, ktiles, mw] stacked on dim0:
        # piece mb holds columns mb*mw..(mb+1)*mw, contiguous per partition
        p = pack(w, ktiles)                       # (128, ktiles, C)
        nmb = p.shape[2] // mw
        return np.ascontiguousarray(
            p.reshape(128, ktiles, nmb, mw).transpose(2, 0, 1, 3)
            .reshape(nmb * 128, ktiles, mw))

    shared = {
        "wihp": _q8(pack_mb(wih_s, KI, 512)),
        "whhp": _q8(pack_mb(whh_s, KH, 512)),
        "whop": _q8(pack_mb(who_s, KH, 128)),
        "wio": _q8(pack(wio_s, KI)),
        "hbc": np.ascontiguousarray(hb.reshape(KH, 128).T),
        "obc": np.ascontiguousarray(ob.reshape(KO, 128).T),
        "bmask": _make_bmask(),
    }
    in_maps = []
    for c in range(N_CORES):
        m = dict(shared)
        xp = pack(np.ascontiguousarray(x[c * BL:(c + 1) * BL].T), KI)
        m["xT"] = _q8(np.ascontiguousarray(
            xp.reshape(128, KI, NCH, CH).transpose(0, 2, 1, 3)
            .reshape(128, NCH * KI, CH)))
        in_maps.append(m)
    return in_maps


def _run(inputs, trace=False, tmpdir=None):
    nc = _get_nc()
    in_maps = _prep_in_maps(inputs)
    res = run_bass_kernel_spmd(nc, in_maps, core_ids=list(range(N_CORES)),
                               trace=trace, tmpdir=tmpdir)
    out = np.empty((B, NO), np.float32)
    for c in range(N_CORES):
        out[c * BL:(c + 1) * BL] = res.results[c]["outT"].T.astype(np.float32)
    return out, res


def kernel(**inputs) -> np.ndarray:
    out, _ = _run(inputs, trace=False)
    return out


if __name__ == "__main__":
    rng = np.random.default_rng(0)
    ins = {
        "inputs": rng.standard_normal((B, NI), dtype=np.float32),
        "input_to_hidden": rng.standard_normal((NH, NI), dtype=np.float32) * 0.02,
        "hidden_to_hidden": rng.standard_normal((NH, NH), dtype=np.float32) * 0.02,
        "output_to_hidden": rng.standard_normal((NH, NO), dtype=np.float32) * 0.02,
        "input_to_output": rng.standard_normal((NO, NI), dtype=np.float32) * 0.02,
        "hidden_to_output": rng.standard_normal((NO, NH), dtype=np.float32) * 0.02,
        "output_to_output": rng.standard_normal((NO, NO), dtype=np.float32) * 0.02,
        "hidden_responses": rng.standard_normal(NH, dtype=np.float32) * 0.1 + 1.0,
        "hidden_biases": rng.standard_normal(NH, dtype=np.float32) * 0.1,
        "output_responses": rng.standard_normal(NO, dtype=np.float32) * 0.1 + 1.0,
        "output_biases": rng.standard_normal(NO, dtype=np.float32) * 0.1,
    }
    out = kernel(**ins)
    print("kernel output", out.shape, out.dtype, out[:2, :4])
